# revision 40
# baseline (speedup 1.0000x reference)
"""LocationAwareAttention Trainium2 Bass kernel, data-parallel over batch on 8 cores.

Reference computation (per batch b):
    conv[t, d]  = sum_k conv_w[d,0,k] * la[t+k-1]            (k=3, zero pad)
    energy      = tanh(q @ Wq.T + v @ Wv.T + conv + conv_b + bias)   # [T, D]
    score[t]    = energy[t] . w_score + b_score   (+ mask -> -1e30)
    sc          = sigmoid(score);  attn = sc / sum(sc)
    context[d]  = sum_t attn[t] * value[t, d]

Kernel strategy (per core: 2 batches):
  * everything PE-facing in bf16 (host converts + pre-transposes), fp32 PSUM.
  * activations loaded in [d, t] layout so the contraction dim (D) is on SBUF
    partitions; energy is produced transposed, which makes bias+tanh a fused
    per-partition ScalarE op and the score reduction a matmul.
  * conv1d folded into the same PSUM accumulation as a K=3 matmul against
    host-prepared shifted copies of last_attn; b_score+mask folded in as a K=1
    matmul; conv_b+bias folded into the tanh activation bias.
  * sigmoid score rows are transposed into [128, 4] columns with K=1
    PE-transpose matmuls; those columns are the stationary operands of the
    context matmuls, which lag one chunk to hide the transpose latency.
  * final L1 normalization (attn and context /= sum sc) is done on host, f32.
"""

import numpy as np
import ml_dtypes
from contextlib import ExitStack

import concourse.bass as bass
import concourse.bacc as bacc
import concourse.tile as tile
from concourse import mybir
from concourse.bass_utils import run_bass_kernel_spmd

BF16 = ml_dtypes.bfloat16

B, T, D = 16, 2048, 1024
NCORES = 8
BL = B // NCORES          # batches per core
TN = 512                  # t-chunk (free dim of projection matmuls)
NCH = T // TN             # chunks per batch
KD = D // 128             # 128-wide tiles along D
NS = TN // 128            # 128-t subchunks per chunk

_CACHE = {}


def _build():
    dt = mybir.dt
    nc = bacc.Bacc("TRN2", target_bir_lowering=False, debug=False,
                   enable_asserts=False, num_devices=NCORES)

    qT = nc.dram_tensor("qT", [BL, D, T], dt.bfloat16, kind="ExternalInput").ap()
    vT = nc.dram_tensor("vT", [BL, D, T], dt.bfloat16, kind="ExternalInput").ap()
    v = nc.dram_tensor("v", [BL, T, D], dt.bfloat16, kind="ExternalInput").ap()
    wqt = nc.dram_tensor("wqt", [D, D], dt.bfloat16, kind="ExternalInput").ap()
    wvt = nc.dram_tensor("wvt", [D, D], dt.bfloat16, kind="ExternalInput").ap()
    convwt = nc.dram_tensor("convwt", [3, D], dt.bfloat16, kind="ExternalInput").ap()
    la3 = nc.dram_tensor("la3", [BL, 3, T], dt.bfloat16, kind="ExternalInput").ap()
    biaspt = nc.dram_tensor("biaspt", [128, KD], dt.float32, kind="ExternalInput").ap()
    wscpt = nc.dram_tensor("wscpt", [128, KD], dt.bfloat16, kind="ExternalInput").ap()
    madd = nc.dram_tensor("madd", [BL, T], dt.bfloat16, kind="ExternalInput").ap()

    ctx_raw = nc.dram_tensor("ctx_raw", [BL, D], dt.float32, kind="ExternalOutput").ap()
    attn_raw = nc.dram_tensor("attn_raw", [BL, T], dt.bfloat16, kind="ExternalOutput").ap()

    TANH = mybir.ActivationFunctionType.Tanh
    SIGM = mybir.ActivationFunctionType.Sigmoid
    COPY = mybir.ActivationFunctionType.Copy

    with tile.TileContext(nc) as tc, ExitStack() as ctx:
        const = ctx.enter_context(tc.tile_pool(name="const", bufs=1))
        xpool = ctx.enter_context(tc.tile_pool(name="xT", bufs=3))
        vpool = ctx.enter_context(tc.tile_pool(name="vnat", bufs=3))
        epool = ctx.enter_context(tc.tile_pool(name="energy", bufs=2))
        spool = ctx.enter_context(tc.tile_pool(name="small", bufs=3))
        bpool = ctx.enter_context(tc.tile_pool(name="perb", bufs=2))
        ps_e = ctx.enter_context(tc.tile_pool(name="ps_energy", bufs=3, space="PSUM"))
        ps_s = ctx.enter_context(tc.tile_pool(name="ps_score", bufs=2, space="PSUM"))
        ps_c = ctx.enter_context(tc.tile_pool(name="ps_ctx", bufs=1, space="PSUM"))
        ps_t = ctx.enter_context(tc.tile_pool(name="ps_tr", bufs=1, space="PSUM"))

        # --- startup: small tensors the first matmul group needs, then the
        # big loads kd-half-interleaved in first-consumption order ---
        convwt_sb = const.tile([3, D], dt.bfloat16, tag="convwt")
        nc.sync.dma_start(out=convwt_sb, in_=convwt)
        la3_0 = bpool.tile([3, T], dt.bfloat16, tag="la3_0", name="la3_0")
        nc.sync.dma_start(out=la3_0, in_=la3[0])
        H = KD // 2
        wq_sb = const.tile([128, KD, D], dt.bfloat16, tag="wq")
        wv_sb = const.tile([128, KD, D], dt.bfloat16, tag="wv")
        xq0 = xpool.tile([128, KD, TN], dt.bfloat16, tag="xq", name="xq0")
        xv0 = xpool.tile([128, KD, TN], dt.bfloat16, tag="xv", name="xv0")
        qT_r = qT[0].rearrange("(kd p) t -> p kd t", p=128)
        vT_r = vT[0].rearrange("(kd p) t -> p kd t", p=128)
        wqt_r = wqt.rearrange("(kd p) e -> p kd e", p=128)
        wvt_r = wvt.rearrange("(kd p) e -> p kd e", p=128)
        for h in range(2):
            ks = slice(h * H, (h + 1) * H)
            nc.sync.dma_start(out=wq_sb[:, ks, :], in_=wqt_r[:, ks, :])
            nc.sync.dma_start(out=xq0[:, ks, :], in_=qT_r[:, ks, 0:TN])
            nc.sync.dma_start(out=wv_sb[:, ks, :], in_=wvt_r[:, ks, :])
            nc.sync.dma_start(out=xv0[:, ks, :], in_=vT_r[:, ks, 0:TN])
        biaspt_sb = const.tile([128, KD], dt.float32, tag="biaspt")
        nc.sync.dma_start(out=biaspt_sb, in_=biaspt)
        wsc_sb = const.tile([128, KD], dt.bfloat16, tag="wsc")
        nc.sync.dma_start(out=wsc_sb, in_=wscpt)
        one_sb = const.tile([1, 1], dt.bfloat16, tag="one")
        nc.vector.memset(one_sb, 1.0)
        la3_1 = bpool.tile([3, T], dt.bfloat16, tag="la3_1", name="la3_1")
        nc.sync.dma_start(out=la3_1, in_=la3[1])
        la3_all = [la3_0, la3_1]
        madd_all = []
        for b in range(BL):
            madd_sb = bpool.tile([1, T], dt.bfloat16, tag=f"madd_{b}", name=f"madd_{b}")
            nc.sync.dma_start(out=madd_sb,
                              in_=madd[b].rearrange("(o t) -> o t", o=1))
            madd_all.append(madd_sb)
        # transpose identity (K=1) for the score-column transpose
        ident1 = const.tile([1, 1], dt.bfloat16, tag="ident1")
        nc.vector.memset(ident1, 1.0)

        for b in range(BL):
            la3_sb = la3_all[b]
            madd_sb = madd_all[b]
            attn_row = bpool.tile([1, T], dt.bfloat16, tag="attn_row")

            cps = [ps_c.tile([1, 512], dt.float32, tag=f"ctx{h}", name=f"cps{h}")
                   for h in range(2)]

            prev = None  # (sct, vn) from previous chunk; ctx matmuls lag one chunk
            for c in range(NCH + 1):
                if c < NCH:
                    t0 = c * TN
                    # host-pretransposed activations, one fused DMA per input
                    if b == 0 and c == 0:
                        xq, xv = xq0, xv0
                    else:
                        xq = xpool.tile([128, KD, TN], dt.bfloat16, tag="xq")
                        nc.sync.dma_start(
                            out=xq,
                            in_=qT[b].rearrange("(kd p) t -> p kd t",
                                                p=128)[:, :, t0:t0 + TN])
                        xv = xpool.tile([128, KD, TN], dt.bfloat16, tag="xv")
                        nc.sync.dma_start(
                            out=xv,
                            in_=vT[b].rearrange("(kd p) t -> p kd t",
                                                p=128)[:, :, t0:t0 + TN])
                    vn = vpool.tile([128, NS, D], dt.bfloat16, tag="vn")
                    nc.gpsimd.dma_start(
                        out=vn,
                        in_=v[b, t0:t0 + TN, :].rearrange("(s p) d -> p s d", p=128))

                    energy = []
                    for e in range(KD):
                        ps = ps_e.tile([128, TN], dt.float32, tag="pse")
                        es = slice(e * 128, (e + 1) * 128)
                        nc.tensor.matmul(ps, convwt_sb[:, es],
                                         la3_sb[:, t0:t0 + TN], start=True, stop=False)
                        # kd-half-interleaved q/v order matches startup arrival
                        for h in range(2):
                            for kd in range(h * H, (h + 1) * H):
                                nc.tensor.matmul(ps, wq_sb[:, kd, es], xq[:, kd, :],
                                                 start=False, stop=False)
                            for kd in range(h * H, (h + 1) * H):
                                nc.tensor.matmul(ps, wv_sb[:, kd, es], xv[:, kd, :],
                                                 start=False,
                                                 stop=(h == 1 and kd == KD - 1))
                        en = epool.tile([128, TN], dt.bfloat16, tag=f"en{e}")
                        nc.scalar.activation(out=en, in_=ps, func=TANH,
                                             bias=biaspt_sb[:, e:e + 1], scale=1.0)
                        energy.append(en)

                    # score row [1, TN]: w_score . energy + (b_score + mask)
                    sps = ps_s.tile([1, TN], dt.float32, tag="score")
                    nc.tensor.matmul(sps, one_sb, madd_sb[:, t0:t0 + TN],
                                     start=True, stop=False)
                    for e in range(KD):
                        nc.tensor.matmul(sps, wsc_sb[:, e:e + 1], energy[e],
                                         start=False, stop=(e == KD - 1))
                    nc.scalar.activation(out=attn_row[:, t0:t0 + TN], in_=sps,
                                         func=SIGM)

                    # transpose sigmoid row into [128, NS] columns on the PE
                    # (K=1 transpose-mode matmuls), then one DVE drain
                    tps = ps_t.tile([128, NS, 2], dt.bfloat16, tag="tps")
                    for s in range(NS):
                        nc.tensor.transpose(
                            tps[:, s, 0:1],
                            attn_row[:, t0 + s * 128:t0 + (s + 1) * 128], ident1)
                    sct = spool.tile([128, NS], dt.bfloat16, tag="sct")
                    nc.vector.tensor_copy(sct, tps[:, :, 0])
                    cur = (sct, vn)
                else:
                    cur = None

                if prev is not None:
                    pc = c - 1
                    sct_p, vn_p = prev
                    for s in range(NS):
                        t128 = pc * NS + s
                        first = (t128 == 0)
                        last = (t128 == T // 128 - 1)
                        nc.tensor.matmul(cps[0], sct_p[:, s:s + 1],
                                         vn_p[:, s, 0:512], start=first, stop=last)
                        nc.tensor.matmul(cps[1], sct_p[:, s:s + 1],
                                         vn_p[:, s, 512:1024], start=first, stop=last)
                prev = cur

            ctx_sb = bpool.tile([1, D], dt.float32, tag="ctx_sb")
            nc.scalar.activation(out=ctx_sb[:, 0:512], in_=cps[0], func=COPY)
            nc.scalar.activation(out=ctx_sb[:, 512:1024], in_=cps[1], func=COPY)
            nc.scalar.dma_start(out=ctx_raw[b].rearrange("(o d) -> o d", o=1),
                                in_=ctx_sb)
            nc.scalar.dma_start(out=attn_raw[b].rearrange("(o t) -> o t", o=1),
                                in_=attn_row)

    nc.compile()
    return nc


def _get_nc():
    if "nc" not in _CACHE:
        _CACHE["nc"] = _build()
    return _CACHE["nc"]


def _prep_inputs(query, value, mask, last_attn, conv_w, conv_b, Wq, Wv,
                 w_score, b_score, bias):
    query = np.ascontiguousarray(np.asarray(query, dtype=np.float32))
    value = np.ascontiguousarray(np.asarray(value, dtype=np.float32))
    mask = np.asarray(mask)
    last_attn = np.asarray(last_attn, dtype=np.float32)
    conv_w = np.asarray(conv_w, dtype=np.float32)
    conv_b = np.asarray(conv_b, dtype=np.float32)
    Wq = np.asarray(Wq, dtype=np.float32)
    Wv = np.asarray(Wv, dtype=np.float32)
    w_score = np.asarray(w_score, dtype=np.float32)
    b_score = np.float32(np.asarray(b_score))
    bias = np.asarray(bias, dtype=np.float32)

    q_bf = query.astype(BF16)
    v_bf = value.astype(BF16)
    qT_bf = np.ascontiguousarray(q_bf.transpose(0, 2, 1))
    vT_bf = np.ascontiguousarray(v_bf.transpose(0, 2, 1))
    wqt = np.ascontiguousarray(Wq.T).astype(BF16)          # [d, e]
    wvt = np.ascontiguousarray(Wv.T).astype(BF16)
    convwt = np.ascontiguousarray(conv_w[:, 0, :].T).astype(BF16)   # [3, D]

    la3 = np.zeros((B, 3, T), np.float32)
    la3[:, 0, 1:] = last_attn[:, :-1]
    la3[:, 1, :] = last_attn
    la3[:, 2, :-1] = last_attn[:, 1:]
    la3 = la3.astype(BF16)

    biasc = bias + conv_b
    biaspt = np.ascontiguousarray(biasc.reshape(KD, 128).T).astype(np.float32)
    wscpt = np.ascontiguousarray(w_score.reshape(KD, 128).T).astype(BF16)

    madd = np.where(mask, np.float32(-1e30), np.float32(0.0)) + b_score
    madd = madd.astype(BF16)                                # [B, T]

    in_maps = []
    for i in range(NCORES):
        sl = slice(i * BL, (i + 1) * BL)
        in_maps.append({
            "qT": qT_bf[sl], "vT": vT_bf[sl], "v": v_bf[sl],
            "wqt": wqt, "wvt": wvt, "convwt": convwt,
            "la3": la3[sl], "biaspt": biaspt, "wscpt": wscpt,
            "madd": madd[sl],
        })
    return in_maps


def _postprocess(results):
    ctx_raw = np.concatenate([results[i]["ctx_raw"] for i in range(NCORES)], axis=0)
    sc = np.concatenate(
        [results[i]["attn_raw"].astype(np.float32) for i in range(NCORES)], axis=0)
    s = sc.sum(axis=-1, keepdims=True)
    attn = sc / s
    context = (ctx_raw / s).astype(np.float32)
    return context.astype(np.float32), attn.astype(np.float32)


def run(inputs, trace=False, **kw):
    """Build (cached), run on 8 cores; returns (context, attn, BassKernelResults)."""
    nc = _get_nc()
    in_maps = _prep_inputs(**inputs)
    res = run_bass_kernel_spmd(nc, in_maps, core_ids=list(range(NCORES)),
                               trace=trace, **kw)
    context, attn = _postprocess(res.results)
    return context, attn, res


def kernel(**inputs):
    context, attn, _ = run(inputs, trace=False)
    return context, attn


# revision 44
# speedup vs baseline: 1.0595x; 1.0595x over previous
"""LocationAwareAttention Trainium2 Bass kernel, data-parallel over batch on 8 cores.

Reference computation (per batch b):
    conv[t, d]  = sum_k conv_w[d,0,k] * la[t+k-1]            (k=3, zero pad)
    energy      = tanh(q @ Wq.T + v @ Wv.T + conv + conv_b + bias)   # [T, D]
    score[t]    = energy[t] . w_score + b_score   (+ mask -> -1e30)
    sc          = sigmoid(score);  attn = sc / sum(sc)
    context[d]  = sum_t attn[t] * value[t, d]

Kernel strategy (per core: 2 batches):
  * everything PE-facing in bf16 (host converts + pre-transposes), fp32 PSUM.
  * activations loaded in [d, t] layout so the contraction dim (D) is on SBUF
    partitions; energy is produced transposed, which makes bias+tanh a fused
    per-partition ScalarE op and the score reduction a matmul.
  * conv1d folded into the same PSUM accumulation as a K=3 matmul against
    host-prepared shifted copies of last_attn; b_score+mask folded in as a K=1
    matmul; conv_b+bias folded into the tanh activation bias.
  * sigmoid score rows are transposed into [128, 4] columns with K=1
    PE-transpose matmuls; those columns are the stationary operands of the
    context matmuls, which lag one chunk to hide the transpose latency.
  * final L1 normalization (attn and context /= sum sc) is done on host, f32.
"""

import numpy as np
import ml_dtypes
from contextlib import ExitStack

import concourse.bass as bass
import concourse.bacc as bacc
import concourse.tile as tile
from concourse import mybir
from concourse.bass_utils import run_bass_kernel_spmd

BF16 = ml_dtypes.bfloat16

B, T, D = 16, 2048, 1024
NCORES = 8
BL = B // NCORES          # batches per core
TN = 512                  # t-chunk (free dim of projection matmuls)
NCH = T // TN             # chunks per batch
KD = D // 128             # 128-wide tiles along D
NS = TN // 128            # 128-t subchunks per chunk

_CACHE = {}


def _build():
    dt = mybir.dt
    nc = bacc.Bacc("TRN2", target_bir_lowering=False, debug=False,
                   enable_asserts=False, num_devices=NCORES)

    qT = nc.dram_tensor("qT", [BL, D, T], dt.bfloat16, kind="ExternalInput").ap()
    vT = nc.dram_tensor("vT", [BL, D, T], dt.bfloat16, kind="ExternalInput").ap()
    v = nc.dram_tensor("v", [BL, T, D], dt.bfloat16, kind="ExternalInput").ap()
    wqt = nc.dram_tensor("wqt", [D, D], dt.bfloat16, kind="ExternalInput").ap()
    wvt = nc.dram_tensor("wvt", [D, D], dt.bfloat16, kind="ExternalInput").ap()
    convwt = nc.dram_tensor("convwt", [3, D], dt.bfloat16, kind="ExternalInput").ap()
    la3 = nc.dram_tensor("la3", [BL, 3, T], dt.bfloat16, kind="ExternalInput").ap()
    biaspt = nc.dram_tensor("biaspt", [128, KD], dt.float32, kind="ExternalInput").ap()
    wscpt = nc.dram_tensor("wscpt", [128, KD], dt.bfloat16, kind="ExternalInput").ap()
    madd = nc.dram_tensor("madd", [BL, T], dt.bfloat16, kind="ExternalInput").ap()

    ctx_raw = nc.dram_tensor("ctx_raw", [BL, D], dt.float32, kind="ExternalOutput").ap()
    attn_raw = nc.dram_tensor("attn_raw", [BL, T], dt.bfloat16, kind="ExternalOutput").ap()

    TANH = mybir.ActivationFunctionType.Tanh
    SIGM = mybir.ActivationFunctionType.Sigmoid
    COPY = mybir.ActivationFunctionType.Copy

    with tile.TileContext(nc) as tc, ExitStack() as ctx:
        const = ctx.enter_context(tc.tile_pool(name="const", bufs=1))
        xpool = ctx.enter_context(tc.tile_pool(name="xT", bufs=3))
        vpool = ctx.enter_context(tc.tile_pool(name="vnat", bufs=3))
        epool = ctx.enter_context(tc.tile_pool(name="energy", bufs=2))
        spool = ctx.enter_context(tc.tile_pool(name="small", bufs=3))
        bpool = ctx.enter_context(tc.tile_pool(name="perb", bufs=2))
        ps_e = ctx.enter_context(tc.tile_pool(name="ps_energy", bufs=4, space="PSUM"))
        ps_s = ctx.enter_context(tc.tile_pool(name="ps_score", bufs=1, space="PSUM"))
        ps_c = ctx.enter_context(tc.tile_pool(name="ps_ctx", bufs=1, space="PSUM"))
        ps_t = ctx.enter_context(tc.tile_pool(name="ps_tr", bufs=1, space="PSUM"))

        # --- startup: small tensors the first matmul group needs, then the
        # big loads kd-half-interleaved in first-consumption order ---
        # conv weights + shifted last_attn replicated into 4 row-group strips
        # (partitions 32g..32g+2) for row-packed concurrent K=3 matmuls
        convwt_sb = const.tile([128, D], dt.bfloat16, tag="convwt")
        for g in range(4):
            nc.sync.dma_start(out=convwt_sb[32 * g:32 * g + 3, :], in_=convwt)
        la3_0 = bpool.tile([128, T], dt.bfloat16, tag="la3_0", name="la3_0")
        for g in range(4):
            nc.sync.dma_start(out=la3_0[32 * g:32 * g + 3, :], in_=la3[0])
        H = KD // 2
        wq_sb = const.tile([128, KD, D], dt.bfloat16, tag="wq")
        wv_sb = const.tile([128, KD, D], dt.bfloat16, tag="wv")
        xq0 = xpool.tile([128, KD, TN], dt.bfloat16, tag="xq", name="xq0")
        xv0 = xpool.tile([128, KD, TN], dt.bfloat16, tag="xv", name="xv0")
        qT_r = qT[0].rearrange("(kd p) t -> p kd t", p=128)
        vT_r = vT[0].rearrange("(kd p) t -> p kd t", p=128)
        wqt_r = wqt.rearrange("(kd p) e -> p kd e", p=128)
        wvt_r = wvt.rearrange("(kd p) e -> p kd e", p=128)
        for h in range(2):
            ks = slice(h * H, (h + 1) * H)
            nc.sync.dma_start(out=wq_sb[:, ks, :], in_=wqt_r[:, ks, :])
            nc.sync.dma_start(out=xq0[:, ks, :], in_=qT_r[:, ks, 0:TN])
            nc.sync.dma_start(out=wv_sb[:, ks, :], in_=wvt_r[:, ks, :])
            nc.sync.dma_start(out=xv0[:, ks, :], in_=vT_r[:, ks, 0:TN])
        biaspt_sb = const.tile([128, KD], dt.float32, tag="biaspt")
        nc.sync.dma_start(out=biaspt_sb, in_=biaspt)
        wsc_sb = const.tile([128, KD], dt.bfloat16, tag="wsc")
        nc.sync.dma_start(out=wsc_sb, in_=wscpt)
        one_sb = const.tile([1, 1], dt.bfloat16, tag="one")
        nc.vector.memset(one_sb, 1.0)
        la3_1 = bpool.tile([128, T], dt.bfloat16, tag="la3_1", name="la3_1")
        for g in range(4):
            nc.sync.dma_start(out=la3_1[32 * g:32 * g + 3, :], in_=la3[1])
        la3_all = [la3_0, la3_1]
        madd_all = []
        for b in range(BL):
            madd_sb = bpool.tile([1, T], dt.bfloat16, tag=f"madd_{b}", name=f"madd_{b}")
            nc.sync.dma_start(out=madd_sb,
                              in_=madd[b].rearrange("(o t) -> o t", o=1))
            madd_all.append(madd_sb)
        # transpose identity (K=1) for the score-column transpose
        ident1 = const.tile([1, 1], dt.bfloat16, tag="ident1")
        nc.vector.memset(ident1, 1.0)

        for b in range(BL):
            la3_sb = la3_all[b]
            madd_sb = madd_all[b]
            attn_row = bpool.tile([1, T], dt.bfloat16, tag="attn_row")

            cps = [ps_c.tile([1, 512], dt.float32, tag=f"ctx{h}", name=f"cps{h}")
                   for h in range(2)]

            prev = None  # (sct, vn) from previous chunk; ctx matmuls lag one chunk
            for c in range(NCH + 1):
                if c < NCH:
                    t0 = c * TN
                    # host-pretransposed activations, one fused DMA per input
                    if b == 0 and c == 0:
                        xq, xv = xq0, xv0
                    else:
                        xq = xpool.tile([128, KD, TN], dt.bfloat16, tag="xq")
                        nc.sync.dma_start(
                            out=xq,
                            in_=qT[b].rearrange("(kd p) t -> p kd t",
                                                p=128)[:, :, t0:t0 + TN])
                        xv = xpool.tile([128, KD, TN], dt.bfloat16, tag="xv")
                        nc.sync.dma_start(
                            out=xv,
                            in_=vT[b].rearrange("(kd p) t -> p kd t",
                                                p=128)[:, :, t0:t0 + TN])
                    vn = vpool.tile([128, NS, D], dt.bfloat16, tag="vn")
                    nc.gpsimd.dma_start(
                        out=vn,
                        in_=v[b, t0:t0 + TN, :].rearrange("(s p) d -> p s d", p=128))

                    energy = []
                    for grp in range(KD // 4):
                        pss4 = []
                        # 4 row-packed concurrent conv matmuls open 4 groups
                        for j in range(4):
                            e = grp * 4 + j
                            ps = ps_e.tile([128, TN], dt.float32, tag="pse",
                                           name=f"pse{e}")
                            es = slice(e * 128, (e + 1) * 128)
                            nc.tensor.matmul(ps, convwt_sb[32 * j:32 * j + 3, es],
                                             la3_sb[32 * j:32 * j + 3, t0:t0 + TN],
                                             start=True, stop=False,
                                             tile_position=(32 * j, 0))
                            pss4.append(ps)
                        for j in range(4):
                            e = grp * 4 + j
                            ps = pss4[j]
                            es = slice(e * 128, (e + 1) * 128)
                            # kd-half-interleaved q/v order matches startup order
                            for h in range(2):
                                for kd in range(h * H, (h + 1) * H):
                                    nc.tensor.matmul(ps, wq_sb[:, kd, es],
                                                     xq[:, kd, :],
                                                     start=False, stop=False)
                                for kd in range(h * H, (h + 1) * H):
                                    nc.tensor.matmul(ps, wv_sb[:, kd, es],
                                                     xv[:, kd, :], start=False,
                                                     stop=(h == 1 and kd == KD - 1))
                            en = epool.tile([128, TN], dt.bfloat16, tag=f"en{e}",
                                            name=f"en{e}")
                            nc.scalar.activation(out=en, in_=ps, func=TANH,
                                                 bias=biaspt_sb[:, e:e + 1],
                                                 scale=1.0)
                            energy.append(en)

                    # score row [1, TN]: w_score . energy + (b_score + mask)
                    sps = ps_s.tile([1, TN], dt.float32, tag="score")
                    nc.tensor.matmul(sps, one_sb, madd_sb[:, t0:t0 + TN],
                                     start=True, stop=False)
                    for e in range(KD):
                        nc.tensor.matmul(sps, wsc_sb[:, e:e + 1], energy[e],
                                         start=False, stop=(e == KD - 1))
                    nc.scalar.activation(out=attn_row[:, t0:t0 + TN], in_=sps,
                                         func=SIGM)

                    # transpose sigmoid row into [128, NS] columns on the PE
                    # (K=1 transpose-mode matmuls), then one DVE drain
                    tps = ps_t.tile([128, NS, 2], dt.bfloat16, tag="tps")
                    for s in range(NS):
                        nc.tensor.transpose(
                            tps[:, s, 0:1],
                            attn_row[:, t0 + s * 128:t0 + (s + 1) * 128], ident1)
                    sct = spool.tile([128, NS], dt.bfloat16, tag="sct")
                    nc.vector.tensor_copy(sct, tps[:, :, 0])
                    cur = (sct, vn)
                else:
                    cur = None

                if prev is not None:
                    pc = c - 1
                    sct_p, vn_p = prev
                    for s in range(NS):
                        t128 = pc * NS + s
                        first = (t128 == 0)
                        last = (t128 == T // 128 - 1)
                        nc.tensor.matmul(cps[0], sct_p[:, s:s + 1],
                                         vn_p[:, s, 0:512], start=first, stop=last)
                        nc.tensor.matmul(cps[1], sct_p[:, s:s + 1],
                                         vn_p[:, s, 512:1024], start=first, stop=last)
                prev = cur

            ctx_sb = bpool.tile([1, D], dt.float32, tag="ctx_sb")
            nc.scalar.activation(out=ctx_sb[:, 0:512], in_=cps[0], func=COPY)
            nc.scalar.activation(out=ctx_sb[:, 512:1024], in_=cps[1], func=COPY)
            nc.scalar.dma_start(out=ctx_raw[b].rearrange("(o d) -> o d", o=1),
                                in_=ctx_sb)
            nc.scalar.dma_start(out=attn_raw[b].rearrange("(o t) -> o t", o=1),
                                in_=attn_row)

    nc.compile()
    return nc


def _get_nc():
    if "nc" not in _CACHE:
        _CACHE["nc"] = _build()
    return _CACHE["nc"]


def _prep_inputs(query, value, mask, last_attn, conv_w, conv_b, Wq, Wv,
                 w_score, b_score, bias):
    query = np.ascontiguousarray(np.asarray(query, dtype=np.float32))
    value = np.ascontiguousarray(np.asarray(value, dtype=np.float32))
    mask = np.asarray(mask)
    last_attn = np.asarray(last_attn, dtype=np.float32)
    conv_w = np.asarray(conv_w, dtype=np.float32)
    conv_b = np.asarray(conv_b, dtype=np.float32)
    Wq = np.asarray(Wq, dtype=np.float32)
    Wv = np.asarray(Wv, dtype=np.float32)
    w_score = np.asarray(w_score, dtype=np.float32)
    b_score = np.float32(np.asarray(b_score))
    bias = np.asarray(bias, dtype=np.float32)

    q_bf = query.astype(BF16)
    v_bf = value.astype(BF16)
    qT_bf = np.ascontiguousarray(q_bf.transpose(0, 2, 1))
    vT_bf = np.ascontiguousarray(v_bf.transpose(0, 2, 1))
    wqt = np.ascontiguousarray(Wq.T).astype(BF16)          # [d, e]
    wvt = np.ascontiguousarray(Wv.T).astype(BF16)
    convwt = np.ascontiguousarray(conv_w[:, 0, :].T).astype(BF16)   # [3, D]

    la3 = np.zeros((B, 3, T), np.float32)
    la3[:, 0, 1:] = last_attn[:, :-1]
    la3[:, 1, :] = last_attn
    la3[:, 2, :-1] = last_attn[:, 1:]
    la3 = la3.astype(BF16)

    biasc = bias + conv_b
    biaspt = np.ascontiguousarray(biasc.reshape(KD, 128).T).astype(np.float32)
    wscpt = np.ascontiguousarray(w_score.reshape(KD, 128).T).astype(BF16)

    madd = np.where(mask, np.float32(-1e30), np.float32(0.0)) + b_score
    madd = madd.astype(BF16)                                # [B, T]

    in_maps = []
    for i in range(NCORES):
        sl = slice(i * BL, (i + 1) * BL)
        in_maps.append({
            "qT": qT_bf[sl], "vT": vT_bf[sl], "v": v_bf[sl],
            "wqt": wqt, "wvt": wvt, "convwt": convwt,
            "la3": la3[sl], "biaspt": biaspt, "wscpt": wscpt,
            "madd": madd[sl],
        })
    return in_maps


def _postprocess(results):
    ctx_raw = np.concatenate([results[i]["ctx_raw"] for i in range(NCORES)], axis=0)
    sc = np.concatenate(
        [results[i]["attn_raw"].astype(np.float32) for i in range(NCORES)], axis=0)
    s = sc.sum(axis=-1, keepdims=True)
    attn = sc / s
    context = (ctx_raw / s).astype(np.float32)
    return context.astype(np.float32), attn.astype(np.float32)


def run(inputs, trace=False, **kw):
    """Build (cached), run on 8 cores; returns (context, attn, BassKernelResults)."""
    nc = _get_nc()
    in_maps = _prep_inputs(**inputs)
    res = run_bass_kernel_spmd(nc, in_maps, core_ids=list(range(NCORES)),
                               trace=trace, **kw)
    context, attn = _postprocess(res.results)
    return context, attn, res


def kernel(**inputs):
    context, attn, _ = run(inputs, trace=False)
    return context, attn
